# revision 2
# baseline (speedup 1.0000x reference)
"""Multi-head causal attention (B=2, T=2048, D=1024, H=16) on 8 trn2 NeuronCores.

Sharding: 8 cores = 2 batches x 4 head-groups (4 heads each). Each core:
  - computes qkv projections for its 4 heads from x[b] (pre-transposed on host),
  - runs masked softmax attention in transposed (k, q) score layout,
  - emits a partial output projection y_part = attn_heads @ w_out[head_rows].
Host sums the 4 partial y per batch.

v2 scheduling notes:
  - scores for the head pair land in one [128, 2, 512] PSUM tile (2 banks) so
    one ScalarE exp covers both heads (halves ACT instruction count).
  - mask multiplies run on the otherwise-idle GpSimd engine (SBUF-only).
  - all dram tensors are host-pre-swizzled so every DMA is 128 contiguous
    partition rows (cheap descriptor generation on the Sync sequencer);
    epilogue/output DMAs are issued from the GpSimd sequencer instead.
  - pair rows interleave (p0-qt, p1-qt) with lazy flushes: k-projection and
    v tiles are pulled in right before the score/attnU step that needs them,
    so PE filler work spreads across the whole exp-paced attention phase.
  - y is written back in bf16 (halves writeback traffic).
"""
import sys
sys.path.insert(0, "/opt/trn_rl_repo")

import numpy as np
import ml_dtypes

import concourse.bass as bass
import concourse.mybir as mybir
import concourse.tile as tile
from concourse import bacc
from concourse.bass_utils import run_bass_kernel_spmd

B, T, D, H, Dh = 2, 2048, 1024, 16, 64
P = 128
QT = 512              # q-tile width (score tile free dim)
NQ = T // QT          # 4
NKT = T // P          # 16
ND = D // P           # 8
HPC = 4               # heads per core
NPAIR = HPC // 2      # head pairs per core
N_CORES = 8

f32 = mybir.dt.float32
bf16 = mybir.dt.bfloat16
CDT = bf16            # compute dtype for matmul operands
NP_CDT = ml_dtypes.bfloat16

FILL_NS = 450         # filler PE-ns drained per attention step


def _block_structure(mask: np.ndarray):
    """Classify maskT (k,q) blocks: per q-tile a list of (kt, pattern_idx|None).

    For each unique partial pattern also derive (w0, m_lo, m_hi): w0 leading
    all-masked columns (exp skipped, memset 0), and [m_lo, m_hi) the column
    range that still needs the mask multiply.
    """
    maskT = (mask != 0).T.astype(np.float32)  # [k, q] visibility
    vis = []
    patterns = []
    meta = []
    pat_index = {}
    for qt in range(NQ):
        row = []
        for kt in range(NKT):
            blk = maskT[kt * P:(kt + 1) * P, qt * QT:(qt + 1) * QT]
            s = blk.sum()
            if s == 0:
                continue
            if s == blk.size:
                row.append((kt, None))
            else:
                key = blk.tobytes()
                if key not in pat_index:
                    pat_index[key] = len(patterns)
                    patterns.append(blk)
                    col_any = blk.any(axis=0)       # column has any visible
                    col_all = blk.all(axis=0)       # column fully visible
                    w0 = int(np.argmax(col_any)) if col_any.any() else QT
                    partial_cols = np.nonzero(col_any & ~col_all)[0]
                    if partial_cols.size:
                        m_lo, m_hi = int(partial_cols[0]), int(partial_cols[-1]) + 1
                    else:
                        m_lo = m_hi = 0
                    meta.append((w0, m_lo, m_hi))
                row.append((kt, pat_index[key]))
        vis.append(row)
    if patterns:
        pm = np.stack(patterns)
    else:
        pm = np.zeros((1, P, QT), np.float32)
    return vis, pm, meta


def _build_program(vis, n_pm, meta=(), compile=True):
    nc = bacc.Bacc() if compile else bass.Bass()
    # host-pre-swizzled layouts: every dram row (dim0) maps to one partition
    # and is contiguous, so DMAs are 128 plain descriptors.
    xc = [nc.declare_dram_parameter(f"x{c}", [P, ND * QT], CDT, isOutput=False)
          for c in range(NQ)]
    wq = nc.declare_dram_parameter("wq", [P, ND * HPC * Dh], CDT, isOutput=False)
    wk = nc.declare_dram_parameter("wk", [P, ND * HPC * Dh], CDT, isOutput=False)
    wv = nc.declare_dram_parameter("wv", [P, ND * HPC * Dh], CDT, isOutput=False)
    wo = nc.declare_dram_parameter("wo", [P, NPAIR * D], CDT, isOutput=False)
    pmask = nc.declare_dram_parameter("pmask", [P, n_pm * 2 * QT], CDT, isOutput=False)
    y = nc.declare_dram_parameter("y", [T, D], CDT, isOutput=True)

    inv_sqrt_dh = 1.0 / float(np.sqrt(Dh))

    with tile.TileContext(nc) as tc:
        with (
            tc.tile_pool(name="persist", bufs=1) as persist,
            tc.tile_pool(name="work", bufs=3) as work,
            tc.tile_pool(name="epi", bufs=2) as epi,
            tc.tile_pool(name="psA", bufs=2, space="PSUM") as psA,
            tc.tile_pool(name="psS", bufs=2, space="PSUM") as psS,
            tc.tile_pool(name="psU", bufs=1, space="PSUM") as psU,
        ):
            # ---- persistent SBUF tensors ----
            xt_sb = persist.tile([P, NQ, ND, QT], CDT, tag="xt")
            wq_sb = persist.tile([P, ND, HPC * Dh], CDT, tag="wq")
            wk_sb = persist.tile([P, ND, HPC * Dh], CDT, tag="wk")
            wv_sb = persist.tile([P, ND, HPC * Dh], CDT, tag="wv")
            wo_sb = persist.tile([P, NPAIR, D], CDT, tag="wo")
            pm_sb = persist.tile([P, n_pm, 2, QT], CDT, tag="pm")
            qT_sb = [persist.tile([P, T], CDT, tag=f"qT{p}", name=f"qT{p}") for p in range(NPAIR)]
            kT_sb = [persist.tile([P, T], CDT, tag=f"kT{p}", name=f"kT{p}") for p in range(NPAIR)]
            at_sb = [persist.tile([P, T], CDT, tag=f"at{p}", name=f"at{p}") for p in range(NPAIR)]
            # v1: per k-tile and head, [128, 128]: for even heads cols 0:64 =
            # v values and cols 64:128 all-ones (for odd heads the reverse),
            # so the attnU matmul emits softmax denominators replicated on the
            # complementary partition half (matmul cost only depends on N).
            v1_sb = persist.tile([P, NKT, HPC, P], CDT, tag="v1")

            # ones blocks (v halves overwritten later); per 4-kt group so the
            # first v evictions don't wait on the whole memset
            for g in range(4):
                nc.gpsimd.memset(v1_sb[:, g * 4:(g + 1) * 4], 1.0)

            nc.sync.dma_start(wv_sb[:], wv.rearrange("p (o e) -> p o e", o=ND))
            nc.sync.dma_start(xt_sb[:, 0], xc[0].rearrange("p (o t) -> p o t", o=ND))
            nc.sync.dma_start(wq_sb[:], wq.rearrange("p (o e) -> p o e", o=ND))
            nc.sync.dma_start(wk_sb[:], wk.rearrange("p (o e) -> p o e", o=ND))
            for c in range(1, NQ):
                nc.sync.dma_start(xt_sb[:, c], xc[c].rearrange("p (o t) -> p o t", o=ND))
            nc.sync.dma_start(wo_sb[:], wo.rearrange("p (r e) -> p r e", r=NPAIR))
            nc.sync.dma_start(pm_sb[:], pmask.rearrange("p (n h q) -> p n h q", n=n_pm, h=2))

            # ---- filler queue: PE-side work interleaved into ACT-paced ----
            # ---- attention steps. entries: (key, pe_cost_ns, thunk)      ----
            fillers = []
            budget = [0.0]

            def drain_budget(extra_ns):
                budget[0] += extra_ns
                while fillers and budget[0] >= fillers[0][1]:
                    key, cost, thunk = fillers.pop(0)
                    budget[0] -= cost
                    thunk()

            def flush(pred):
                """Emit and remove exactly the queued fillers matching pred."""
                keep = []
                for ent in fillers:
                    if pred(ent[0]):
                        ent[2]()
                    else:
                        keep.append(ent)
                fillers[:] = keep

            # ---- phase A: v = x @ wv for one k-tile ----
            def emit_v(tt):
                ps_v = psA.tile([P, QT], f32, tag="psA", name=f"psv{tt}")
                c, t0 = tt // 4, (tt % 4) * P
                for dt in range(ND):
                    nc.tensor.matmul(
                        ps_v[:, :HPC * Dh],
                        xt_sb[:, c, dt, t0:t0 + P],
                        wv_sb[:, dt, :],
                        start=(dt == 0),
                        stop=(dt == ND - 1),
                    )
                ps_vh = ps_v[:, :HPC * Dh].rearrange("p (h e) -> p h e", h=HPC)
                nc.vector.tensor_copy(v1_sb[:, tt, 0::2, 0:Dh], ps_vh[:, 0::2])
                nc.vector.tensor_copy(v1_sb[:, tt, 1::2, Dh:P], ps_vh[:, 1::2])

            # ---- phase A: qT / kT projection unit for (pair, tensor, nt) ----
            def make_qk(p, which, nt):
                w_sb = wq_sb if which == "q" else wk_sb
                out_sb = qT_sb[p] if which == "q" else kT_sb[p]

                def go():
                    ps = psA.tile([P, QT], f32, tag="psA", name=f"psqk{p}{which}{nt}")
                    for dt in range(ND):
                        nc.tensor.matmul(
                            ps,
                            w_sb[:, dt, p * P:(p + 1) * P],
                            xt_sb[:, nt, dt, :],
                            start=(dt == 0),
                            stop=(dt == ND - 1),
                        )
                    nc.vector.tensor_copy(out_sb[:, nt * QT:(nt + 1) * QT], ps)
                return go

            # ---- phase C: out-projection for one t-tile/half (as filler) ----
            def make_outproj(tt, half):
                def go():
                    ps_y = psA.tile([P, QT], f32, tag="psA", name=f"psy{tt}_{half}")
                    for p in range(NPAIR):
                        nc.tensor.matmul(
                            ps_y[:],
                            at_sb[p][:, tt * P:(tt + 1) * P],
                            wo_sb[:, p, half * QT:(half + 1) * QT],
                            start=(p == 0),
                            stop=(p == NPAIR - 1),
                        )
                    ysb = work.tile([P, QT], CDT, tag="y", name=f"y{tt}_{half}")
                    nc.vector.tensor_copy(ysb[:], ps_y[:])
                    nc.gpsimd.dma_start(
                        y[tt * P:(tt + 1) * P, half * QT:(half + 1) * QT], ysb[:])
                return go

            # ---- phase B: attention row (pair p, q-tile qt) ----
            def attention_row(p, qt):
                row = vis[qt]
                flush(lambda k: k[0] == "qk" and k[1] == p and k[2] == "q" and k[3] == qt)
                if not row:
                    nc.vector.memset(at_sb[p][:, qt * QT:(qt + 1) * QT], 0.0)
                    return
                ps_u = psU.tile([P, 2, QT], f32, tag="u", name=f"u_{p}_{qt}")
                es_q = []

                def emit_scores(j):
                    kt, pidx = row[j]
                    flush(lambda k: k[0] == "qk" and k[1] == p and k[2] == "k"
                          and k[3] <= kt // 4)
                    w0 = 0 if pidx is None else meta[pidx][0]
                    if j == 0:
                        w0 = 0  # first accumulation must set has_written
                    ps_s = psS.tile([P, 2, QT], f32, tag="s", name=f"s_{p}_{qt}_{kt}")
                    for h in range(2):
                        base = h * Dh
                        nc.tensor.matmul(
                            ps_s[:, h, w0:QT],
                            kT_sb[p][base:base + Dh, kt * P:(kt + 1) * P],
                            qT_sb[p][base:base + Dh, qt * QT + w0:(qt + 1) * QT],
                            start=True,
                            stop=True,
                            tile_position=(base, 0),
                        )
                    es = work.tile([P, 2, QT], CDT, tag="es", name=f"es_{p}_{qt}_{kt}")
                    nc.scalar.activation(
                        es[:, :, w0:QT], ps_s[:, :, w0:QT],
                        mybir.ActivationFunctionType.Exp,
                        scale=inv_sqrt_dh,
                    )
                    if pidx is not None:
                        _w0, m_lo, m_hi = meta[pidx]
                        m_lo = min(m_lo, w0)  # w0 forced to 0 on j==0
                        if m_hi > m_lo:
                            nc.gpsimd.tensor_mul(
                                es[:, :, m_lo:m_hi], es[:, :, m_lo:m_hi],
                                pm_sb[:, pidx, :, m_lo:m_hi],
                            )
                    es_q.append((es, w0))

                def emit_attnu(j):
                    kt, _ = row[j]
                    flush(lambda k: k[0] == "v" and k[1] == kt)
                    es, w0 = es_q[j]
                    for h in range(2):
                        nc.tensor.matmul(
                            ps_u[:, h, w0:QT],
                            v1_sb[:, kt, 2 * p + h, :],
                            es[:, h, w0:QT],
                            start=(j == 0),
                            stop=(j == len(row) - 1),
                        )

                emit_scores(0)
                if len(row) > 1:
                    emit_scores(1)
                for j in range(len(row)):
                    drain_budget(FILL_NS)
                    if j + 2 < len(row):
                        emit_scores(j + 2)
                    emit_attnu(j)

                for h in range(2):
                    # ps_u[:, h]: for h==0 partitions 0:64 = attn, 64:128 =
                    # denominators replicated (ones half of v1); for h==1 the
                    # reverse. The custom-DVE reciprocal only works at base
                    # partition 0 on HW, so stage denominators at 0:64.
                    a_sl = slice(0, Dh) if h == 0 else slice(Dh, P)
                    s_sl = slice(Dh, P) if h == 0 else slice(0, Dh)
                    sums = epi.tile([P, QT], f32, tag="sums", name=f"sums{h}_{p}_{qt}")
                    nc.vector.tensor_copy(sums[s_sl, :], ps_u[s_sl, h, :])
                    if h == 0:
                        sums2 = epi.tile([P, QT], f32, tag="sums2", name=f"s2_{h}_{p}_{qt}")
                        nc.gpsimd.dma_start(sums2[0:Dh, :], sums[Dh:P, :])
                        rsrc = sums2
                    else:
                        rsrc = sums
                    rep = epi.tile([P, QT], f32, tag="rep", name=f"rep{h}_{p}_{qt}")
                    nc.vector.reciprocal_approx_fast(rep[0:Dh, :], rsrc[0:Dh, :])
                    if h == 0:
                        mul_in1 = rep[0:Dh, :]
                    else:
                        rep2 = epi.tile([P, QT], f32, tag="rep2", name=f"rp2_{h}_{p}_{qt}")
                        nc.gpsimd.dma_start(rep2[Dh:P, :], rep[0:Dh, :])
                        mul_in1 = rep2[Dh:P, :]
                    nc.vector.tensor_mul(
                        at_sb[p][h * Dh:(h + 1) * Dh, qt * QT:(qt + 1) * QT],
                        ps_u[a_sl, h, :],
                        mul_in1,
                    )
                if p == NPAIR - 1:
                    fillers.extend(
                        (("op", tt, half), 500, make_outproj(tt, half))
                        for tt in range(qt * (QT // P), (qt + 1) * (QT // P))
                        for half in range(2)
                    )

            # inline prologue: v for the first q-tile's k range
            for tt in range(4):
                emit_v(tt)
            fillers.extend((("v", tt), 860, (lambda tt=tt: emit_v(tt)))
                           for tt in range(4, NKT))
            for p in range(NPAIR):
                fillers.extend((("qk", p, which, nt), 1750, make_qk(p, which, nt))
                               for which in ("q", "k") for nt in range(NQ))

            for qt in range(NQ):
                for p in range(NPAIR):
                    attention_row(p, qt)
            flush(lambda k: True)
    if compile:
        nc.compile()
    return nc


def _host_inputs(x, mask, w_qkv, w_out):
    vis, pm, meta = _block_structure(np.asarray(mask))
    # pmask duplicated along an h axis so one multiply covers the merged
    # two-head es tile: [P, n_pm, 2, QT]
    pm_h = np.broadcast_to(
        pm.transpose(1, 0, 2)[:, :, None, :], (P, pm.shape[0], 2, QT)
    ).reshape(P, -1).astype(NP_CDT)
    pm_h = np.ascontiguousarray(pm_h)
    wq_f, wk_f, wv_f = np.split(np.asarray(w_qkv, np.float32), 3, axis=1)

    def swz_w(w):  # [1024, E] -> [128, 8*E] with row (o*128+p) at (p, o)
        e = w.shape[1]
        return np.ascontiguousarray(
            w.reshape(ND, P, e).transpose(1, 0, 2).reshape(P, ND * e)
        ).astype(NP_CDT)

    in_maps = []
    for core in range(N_CORES):
        b = core // 4
        g = core % 4
        cols = slice(g * HPC * Dh, (g + 1) * HPC * Dh)
        xT = np.asarray(x[b], np.float32).T  # [D, T]
        xcs = {}
        for c in range(NQ):
            xcs[f"x{c}"] = np.ascontiguousarray(
                xT[:, c * QT:(c + 1) * QT].reshape(ND, P, QT)
                .transpose(1, 0, 2).reshape(P, ND * QT)
            ).astype(NP_CDT)
        wo_c = np.asarray(w_out, np.float32)[cols, :]  # [256, 1024]
        wo_s = np.ascontiguousarray(
            wo_c.reshape(NPAIR, P, D).transpose(1, 0, 2).reshape(P, NPAIR * D)
        ).astype(NP_CDT)
        in_maps.append({
            **xcs,
            "wq": swz_w(wq_f[:, cols]),
            "wk": swz_w(wk_f[:, cols]),
            "wv": swz_w(wv_f[:, cols]),
            "wo": wo_s,
            "pmask": pm_h,
        })
    return vis, pm, meta, in_maps


def run(x, mask, w_qkv, w_out, trace=False):
    import os
    vis, pm, meta, in_maps = _host_inputs(x, mask, w_qkv, w_out)
    nc = _build_program(vis, pm.shape[0], meta)
    if not trace:
        # An inherited BASS_TRACE=1 would pull in NTFF profiling hooks that
        # may not exist in this environment; force tracing off.
        os.environ["BASS_NEVER_TRACE"] = "1"
    else:
        os.environ.pop("BASS_NEVER_TRACE", None)
    res = run_bass_kernel_spmd(nc, in_maps, core_ids=list(range(N_CORES)), trace=trace)
    parts = [res.results[i]["y"].astype(np.float32) for i in range(N_CORES)]
    out = np.stack([
        parts[0] + parts[1] + parts[2] + parts[3],
        parts[4] + parts[5] + parts[6] + parts[7],
    ]).astype(np.float32)
    return out, res


def kernel(x, mask, w_qkv, w_out):
    out, _ = run(x, mask, w_qkv, w_out, trace=False)
    return out


# revision 4
# speedup vs baseline: 1.0991x; 1.0991x over previous
"""Multi-head causal attention (B=2, T=2048, D=1024, H=16) on 8 trn2 NeuronCores.

Sharding: 8 cores = 2 batches x 4 head-groups (4 heads each). Each core:
  - computes qkv projections for its 4 heads from x[b] (pre-transposed on host),
  - runs masked softmax attention in transposed (k, q) score layout,
  - emits a partial output projection y_part = attn_heads @ w_out[head_rows].
Host sums the 4 partial y per batch.

v2 scheduling notes:
  - scores for the head pair land in one [128, 2, 512] PSUM tile (2 banks) so
    one ScalarE exp covers both heads (halves ACT instruction count).
  - mask multiplies run on the otherwise-idle GpSimd engine (SBUF-only).
  - all dram tensors are host-pre-swizzled so every DMA is 128 contiguous
    partition rows (cheap descriptor generation on the Sync sequencer);
    epilogue/output DMAs are issued from the GpSimd sequencer instead.
  - pair rows interleave (p0-qt, p1-qt) with lazy flushes: k-projection and
    v tiles are pulled in right before the score/attnU step that needs them,
    so PE filler work spreads across the whole exp-paced attention phase.
  - y is written back in bf16 (halves writeback traffic).
"""
import sys
sys.path.insert(0, "/opt/trn_rl_repo")

import numpy as np
import ml_dtypes

import concourse.bass as bass
import concourse.mybir as mybir
import concourse.tile as tile
from concourse import bacc
from concourse.bass_utils import run_bass_kernel_spmd

B, T, D, H, Dh = 2, 2048, 1024, 16, 64
P = 128
QT = 512              # q-tile width (score tile free dim)
NQ = T // QT          # 4
NKT = T // P          # 16
ND = D // P           # 8
HPC = 4               # heads per core
NPAIR = HPC // 2      # head pairs per core
N_CORES = 8

f32 = mybir.dt.float32
bf16 = mybir.dt.bfloat16
CDT = bf16            # compute dtype for matmul operands
NP_CDT = ml_dtypes.bfloat16

FILL_NS = 450         # filler PE-ns drained per attention step


def _block_structure(mask: np.ndarray):
    """Classify maskT (k,q) blocks: per q-tile a list of (kt, pattern_idx|None).

    For each unique partial pattern also derive (w0, m_lo, m_hi): w0 leading
    all-masked columns (exp skipped, memset 0), and [m_lo, m_hi) the column
    range that still needs the mask multiply.
    """
    maskT = (mask != 0).T.astype(np.float32)  # [k, q] visibility
    vis = []
    patterns = []
    meta = []
    pat_index = {}
    for qt in range(NQ):
        row = []
        for kt in range(NKT):
            blk = maskT[kt * P:(kt + 1) * P, qt * QT:(qt + 1) * QT]
            s = blk.sum()
            if s == 0:
                continue
            if s == blk.size:
                row.append((kt, None))
            else:
                key = blk.tobytes()
                if key not in pat_index:
                    pat_index[key] = len(patterns)
                    patterns.append(blk)
                    col_any = blk.any(axis=0)       # column has any visible
                    col_all = blk.all(axis=0)       # column fully visible
                    w0 = int(np.argmax(col_any)) if col_any.any() else QT
                    partial_cols = np.nonzero(col_any & ~col_all)[0]
                    if partial_cols.size:
                        m_lo, m_hi = int(partial_cols[0]), int(partial_cols[-1]) + 1
                    else:
                        m_lo = m_hi = 0
                    meta.append((w0, m_lo, m_hi))
                row.append((kt, pat_index[key]))
        vis.append(row)
    if patterns:
        pm = np.stack(patterns)
    else:
        pm = np.zeros((1, P, QT), np.float32)
    return vis, pm, meta


def _build_program(vis, n_pm, meta=(), compile=True):
    nc = bacc.Bacc() if compile else bass.Bass()
    # host-pre-swizzled layouts: every dram row (dim0) maps to one partition
    # and is contiguous, so DMAs are 128 plain descriptors.
    xc = [nc.declare_dram_parameter(f"x{c}", [P, ND * QT], CDT, isOutput=False)
          for c in range(NQ)]
    wq = nc.declare_dram_parameter("wq", [P, ND * HPC * Dh], CDT, isOutput=False)
    wk = nc.declare_dram_parameter("wk", [P, ND * HPC * Dh], CDT, isOutput=False)
    wv = nc.declare_dram_parameter("wv", [P, ND * HPC * Dh], CDT, isOutput=False)
    wo = nc.declare_dram_parameter("wo", [P, NPAIR * D], CDT, isOutput=False)
    pmask = nc.declare_dram_parameter("pmask", [P, n_pm * 2 * QT], CDT, isOutput=False)
    y = nc.declare_dram_parameter("y", [T, D], CDT, isOutput=True)

    inv_sqrt_dh = 1.0 / float(np.sqrt(Dh))

    with tile.TileContext(nc) as tc:
        with (
            tc.tile_pool(name="persist", bufs=1) as persist,
            tc.tile_pool(name="work", bufs=3) as work,
            tc.tile_pool(name="epi", bufs=2) as epi,
            tc.tile_pool(name="psA", bufs=2, space="PSUM") as psA,
            tc.tile_pool(name="psS", bufs=2, space="PSUM") as psS,
            tc.tile_pool(name="psU", bufs=1, space="PSUM") as psU,
        ):
            # ---- persistent SBUF tensors ----
            xt_sb = persist.tile([P, NQ, ND, QT], CDT, tag="xt")
            wq_sb = persist.tile([P, ND, HPC * Dh], CDT, tag="wq")
            wk_sb = persist.tile([P, ND, HPC * Dh], CDT, tag="wk")
            wv_sb = persist.tile([P, ND, HPC * Dh], CDT, tag="wv")
            wo_sb = persist.tile([P, NPAIR, D], CDT, tag="wo")
            pm_sb = persist.tile([P, n_pm, 2, QT], CDT, tag="pm")
            qT_sb = [persist.tile([P, T], CDT, tag=f"qT{p}", name=f"qT{p}") for p in range(NPAIR)]
            kT_sb = [persist.tile([P, T], CDT, tag=f"kT{p}", name=f"kT{p}") for p in range(NPAIR)]
            at_sb = [persist.tile([P, T], CDT, tag=f"at{p}", name=f"at{p}") for p in range(NPAIR)]
            # v1: per k-tile and head, [128, 128]: for even heads cols 0:64 =
            # v values and cols 64:128 all-ones (for odd heads the reverse),
            # so the attnU matmul emits softmax denominators replicated on the
            # complementary partition half (matmul cost only depends on N).
            v1_sb = persist.tile([P, NKT, HPC, P], CDT, tag="v1")

            # ones blocks (v halves overwritten later); per 4-kt group so the
            # first v evictions don't wait on the whole memset
            for g in range(4):
                nc.gpsimd.memset(v1_sb[:, g * 4:(g + 1) * 4], 1.0)

            nc.sync.dma_start(wv_sb[:], wv.rearrange("p (o e) -> p o e", o=ND))
            nc.sync.dma_start(xt_sb[:, 0], xc[0].rearrange("p (o t) -> p o t", o=ND))
            nc.sync.dma_start(wq_sb[:], wq.rearrange("p (o e) -> p o e", o=ND))
            nc.sync.dma_start(wk_sb[:], wk.rearrange("p (o e) -> p o e", o=ND))
            for c in range(1, NQ):
                nc.sync.dma_start(xt_sb[:, c], xc[c].rearrange("p (o t) -> p o t", o=ND))
            nc.sync.dma_start(wo_sb[:], wo.rearrange("p (r e) -> p r e", r=NPAIR))
            nc.sync.dma_start(pm_sb[:], pmask.rearrange("p (n h q) -> p n h q", n=n_pm, h=2))

            # ---- filler queue: PE-side work interleaved into ACT-paced ----
            # ---- attention steps. entries: (key, pe_cost_ns, thunk)      ----
            fillers = []
            budget = [0.0]

            def drain_budget(extra_ns):
                budget[0] += extra_ns
                while fillers and budget[0] >= fillers[0][1]:
                    key, cost, thunk = fillers.pop(0)
                    budget[0] -= cost
                    thunk()

            def flush(pred):
                """Emit and remove exactly the queued fillers matching pred."""
                keep = []
                for ent in fillers:
                    if pred(ent[0]):
                        ent[2]()
                    else:
                        keep.append(ent)
                fillers[:] = keep

            # ---- phase A: v = x @ wv for one k-tile ----
            def emit_v(tt):
                ps_v = psA.tile([P, QT], f32, tag="psA", name=f"psv{tt}")
                c, t0 = tt // 4, (tt % 4) * P
                for dt in range(ND):
                    nc.tensor.matmul(
                        ps_v[:, :HPC * Dh],
                        xt_sb[:, c, dt, t0:t0 + P],
                        wv_sb[:, dt, :],
                        start=(dt == 0),
                        stop=(dt == ND - 1),
                    )
                ps_vh = ps_v[:, :HPC * Dh].rearrange("p (h e) -> p h e", h=HPC)
                nc.vector.tensor_copy(v1_sb[:, tt, 0::2, 0:Dh], ps_vh[:, 0::2])
                nc.vector.tensor_copy(v1_sb[:, tt, 1::2, Dh:P], ps_vh[:, 1::2])

            # ---- phase A: qT / kT projection unit for (pair, tensor, nt) ----
            def make_qk(p, which, nt):
                w_sb = wq_sb if which == "q" else wk_sb
                out_sb = qT_sb[p] if which == "q" else kT_sb[p]

                def go():
                    ps = psA.tile([P, QT], f32, tag="psA", name=f"psqk{p}{which}{nt}")
                    for dt in range(ND):
                        nc.tensor.matmul(
                            ps,
                            w_sb[:, dt, p * P:(p + 1) * P],
                            xt_sb[:, nt, dt, :],
                            start=(dt == 0),
                            stop=(dt == ND - 1),
                        )
                    nc.vector.tensor_copy(out_sb[:, nt * QT:(nt + 1) * QT], ps)
                return go

            # ---- phase C: out-projection for one t-tile/half (as filler) ----
            def make_outproj(tt, half):
                def go():
                    ps_y = psA.tile([P, QT], f32, tag="psA", name=f"psy{tt}_{half}")
                    for p in range(NPAIR):
                        nc.tensor.matmul(
                            ps_y[:],
                            at_sb[p][:, tt * P:(tt + 1) * P],
                            wo_sb[:, p, half * QT:(half + 1) * QT],
                            start=(p == 0),
                            stop=(p == NPAIR - 1),
                        )
                    ysb = work.tile([P, QT], CDT, tag="y", name=f"y{tt}_{half}")
                    nc.vector.tensor_copy(ysb[:], ps_y[:])
                    nc.sync.dma_start(
                        y[tt * P:(tt + 1) * P, half * QT:(half + 1) * QT], ysb[:])
                return go

            # ---- phase B: attention row (pair p, q-tile qt) ----
            def attention_row(p, qt):
                row = vis[qt]
                flush(lambda k: k[0] == "qk" and k[1] == p and k[2] == "q" and k[3] == qt)
                if not row:
                    nc.vector.memset(at_sb[p][:, qt * QT:(qt + 1) * QT], 0.0)
                    return
                ps_u = psU.tile([P, 2, QT], f32, tag="u", name=f"u_{p}_{qt}")
                es_q = []

                def emit_scores(j):
                    kt, pidx = row[j]
                    flush(lambda k: k[0] == "qk" and k[1] == p and k[2] == "k"
                          and k[3] <= kt // 4)
                    w0 = 0 if pidx is None else meta[pidx][0]
                    if j == 0:
                        w0 = 0  # first accumulation must set has_written
                    ps_s = psS.tile([P, 2, QT], f32, tag="s", name=f"s_{p}_{qt}_{kt}")
                    for h in range(2):
                        base = h * Dh
                        nc.tensor.matmul(
                            ps_s[:, h, w0:QT],
                            kT_sb[p][base:base + Dh, kt * P:(kt + 1) * P],
                            qT_sb[p][base:base + Dh, qt * QT + w0:(qt + 1) * QT],
                            start=True,
                            stop=True,
                            tile_position=(base, 0),
                        )
                    es = work.tile([P, 2, QT], CDT, tag="es", name=f"es_{p}_{qt}_{kt}")
                    nc.scalar.activation(
                        es[:, :, w0:QT], ps_s[:, :, w0:QT],
                        mybir.ActivationFunctionType.Exp,
                        scale=inv_sqrt_dh,
                    )
                    if pidx is not None:
                        _w0, m_lo, m_hi = meta[pidx]
                        m_lo = min(m_lo, w0)  # w0 forced to 0 on j==0
                        if m_hi > m_lo:
                            nc.gpsimd.tensor_mul(
                                es[:, :, m_lo:m_hi], es[:, :, m_lo:m_hi],
                                pm_sb[:, pidx, :, m_lo:m_hi],
                            )
                    es_q.append((es, w0))

                def emit_attnu(j):
                    kt, _ = row[j]
                    flush(lambda k: k[0] == "v" and k[1] == kt)
                    es, w0 = es_q[j]
                    for h in range(2):
                        nc.tensor.matmul(
                            ps_u[:, h, w0:QT],
                            v1_sb[:, kt, 2 * p + h, :],
                            es[:, h, w0:QT],
                            start=(j == 0),
                            stop=(j == len(row) - 1),
                        )

                emit_scores(0)
                if len(row) > 1:
                    emit_scores(1)
                for j in range(len(row)):
                    drain_budget(FILL_NS)
                    if j + 2 < len(row):
                        emit_scores(j + 2)
                    emit_attnu(j)

                # Stage both ps_u halves to SBUF with one ScalarE ACT copy per
                # head (Copy shares the exp ACT table -> no table reload).
                # This releases the PSUM banks ~1.3us after the row's last
                # attnU, so the next row's attnU(0) (psU bufs=1 WAR) never
                # stalls the PE on the divide chain below.
                stg = [epi.tile([P, QT], f32, tag=f"stg{h}", name=f"stg{h}_{p}_{qt}")
                       for h in range(2)]
                for h in range(2):
                    nc.scalar.copy(stg[h][:], ps_u[:, h, :])
                for h in range(2):
                    # staged[:, h]: for h==0 partitions 0:64 = attn, 64:128 =
                    # denominators replicated (ones half of v1); for h==1 the
                    # reverse. The custom-DVE reciprocal only works at base
                    # partition 0 on HW, so denominators go to partitions 0:64.
                    a_sl = slice(0, Dh) if h == 0 else slice(Dh, P)
                    if h == 0:
                        sums2 = epi.tile([P, QT], f32, tag="sums2", name=f"s2_{h}_{p}_{qt}")
                        nc.sync.dma_start(sums2[0:Dh, :], stg[0][Dh:P, :])
                        rsrc = sums2
                    else:
                        rsrc = stg[1]
                    rep = epi.tile([P, QT], f32, tag="rep", name=f"rep{h}_{p}_{qt}")
                    nc.vector.reciprocal_approx_fast(rep[0:Dh, :], rsrc[0:Dh, :])
                    if h == 0:
                        mul_in1 = rep[0:Dh, :]
                    else:
                        rep2 = epi.tile([P, QT], f32, tag="rep2", name=f"rp2_{h}_{p}_{qt}")
                        nc.sync.dma_start(rep2[Dh:P, :], rep[0:Dh, :])
                        mul_in1 = rep2[Dh:P, :]
                    nc.vector.tensor_mul(
                        at_sb[p][h * Dh:(h + 1) * Dh, qt * QT:(qt + 1) * QT],
                        stg[h][a_sl, :],
                        mul_in1,
                    )
                if p == NPAIR - 1:
                    fillers.extend(
                        (("op", tt, half), 500, make_outproj(tt, half))
                        for tt in range(qt * (QT // P), (qt + 1) * (QT // P))
                        for half in range(2)
                    )

            # inline prologue: v for the first q-tile's k range
            for tt in range(4):
                emit_v(tt)
            fillers.extend((("v", tt), 860, (lambda tt=tt: emit_v(tt)))
                           for tt in range(4, NKT))
            for p in range(NPAIR):
                fillers.extend((("qk", p, which, nt), 1750, make_qk(p, which, nt))
                               for which in ("q", "k") for nt in range(NQ))

            for qt in range(NQ):
                for p in range(NPAIR):
                    attention_row(p, qt)
            flush(lambda k: True)
    if compile:
        nc.compile()
    return nc


def _host_inputs(x, mask, w_qkv, w_out):
    vis, pm, meta = _block_structure(np.asarray(mask))
    # pmask duplicated along an h axis so one multiply covers the merged
    # two-head es tile: [P, n_pm, 2, QT]
    pm_h = np.broadcast_to(
        pm.transpose(1, 0, 2)[:, :, None, :], (P, pm.shape[0], 2, QT)
    ).reshape(P, -1).astype(NP_CDT)
    pm_h = np.ascontiguousarray(pm_h)
    wq_f, wk_f, wv_f = np.split(np.asarray(w_qkv, np.float32), 3, axis=1)

    def swz_w(w):  # [1024, E] -> [128, 8*E] with row (o*128+p) at (p, o)
        e = w.shape[1]
        return np.ascontiguousarray(
            w.reshape(ND, P, e).transpose(1, 0, 2).reshape(P, ND * e)
        ).astype(NP_CDT)

    in_maps = []
    for core in range(N_CORES):
        b = core // 4
        g = core % 4
        cols = slice(g * HPC * Dh, (g + 1) * HPC * Dh)
        xT = np.asarray(x[b], np.float32).T  # [D, T]
        xcs = {}
        for c in range(NQ):
            xcs[f"x{c}"] = np.ascontiguousarray(
                xT[:, c * QT:(c + 1) * QT].reshape(ND, P, QT)
                .transpose(1, 0, 2).reshape(P, ND * QT)
            ).astype(NP_CDT)
        wo_c = np.asarray(w_out, np.float32)[cols, :]  # [256, 1024]
        wo_s = np.ascontiguousarray(
            wo_c.reshape(NPAIR, P, D).transpose(1, 0, 2).reshape(P, NPAIR * D)
        ).astype(NP_CDT)
        in_maps.append({
            **xcs,
            "wq": swz_w(wq_f[:, cols]),
            "wk": swz_w(wk_f[:, cols]),
            "wv": swz_w(wv_f[:, cols]),
            "wo": wo_s,
            "pmask": pm_h,
        })
    return vis, pm, meta, in_maps


def run(x, mask, w_qkv, w_out, trace=False):
    import os
    vis, pm, meta, in_maps = _host_inputs(x, mask, w_qkv, w_out)
    nc = _build_program(vis, pm.shape[0], meta)
    if not trace:
        # An inherited BASS_TRACE=1 would pull in NTFF profiling hooks that
        # may not exist in this environment; force tracing off.
        os.environ["BASS_NEVER_TRACE"] = "1"
    else:
        os.environ.pop("BASS_NEVER_TRACE", None)
    res = run_bass_kernel_spmd(nc, in_maps, core_ids=list(range(N_CORES)), trace=trace)
    parts = [res.results[i]["y"].astype(np.float32) for i in range(N_CORES)]
    out = np.stack([
        parts[0] + parts[1] + parts[2] + parts[3],
        parts[4] + parts[5] + parts[6] + parts[7],
    ]).astype(np.float32)
    return out, res


def kernel(x, mask, w_qkv, w_out):
    out, _ = run(x, mask, w_qkv, w_out, trace=False)
    return out


# revision 14
# speedup vs baseline: 1.2980x; 1.1809x over previous
"""Multi-head causal attention (B=2, T=2048, D=1024, H=16) on 8 trn2 NeuronCores.

Sharding: 8 cores = 2 batches x 4 head-groups (4 heads each). Each core:
  - computes qkv projections for its 4 heads from x[b] (pre-transposed on host),
  - runs masked softmax attention in transposed (k, q) score layout,
  - emits a partial output projection y_part = attn_heads @ w_out[head_rows].
Host sums the 4 partial y per batch.

v2 scheduling notes:
  - scores for the head pair land in one [128, 2, 512] PSUM tile (2 banks) so
    one ScalarE exp covers both heads (halves ACT instruction count).
  - mask multiplies run on the otherwise-idle GpSimd engine (SBUF-only).
  - all dram tensors are host-pre-swizzled so every DMA is 128 contiguous
    partition rows (cheap descriptor generation on the Sync sequencer);
    epilogue/output DMAs are issued from the GpSimd sequencer instead.
  - pair rows interleave (p0-qt, p1-qt) with lazy flushes: k-projection and
    v tiles are pulled in right before the score/attnU step that needs them,
    so PE filler work spreads across the whole exp-paced attention phase.
  - y is written back in bf16 (halves writeback traffic).
"""
import sys
sys.path.insert(0, "/opt/trn_rl_repo")

import numpy as np
import ml_dtypes

import concourse.bass as bass
import concourse.mybir as mybir
import concourse.tile as tile
from concourse import bacc
from concourse.bass_utils import run_bass_kernel_spmd

B, T, D, H, Dh = 2, 2048, 1024, 16, 64
P = 128
QT = 512              # q-tile width (score tile free dim)
NQ = T // QT          # 4
NKT = T // P          # 16
ND = D // P           # 8
HPC = 4               # heads per core
NPAIR = HPC // 2      # head pairs per core
N_CORES = 8

f32 = mybir.dt.float32
bf16 = mybir.dt.bfloat16
CDT = bf16            # compute dtype for matmul operands
NP_CDT = ml_dtypes.bfloat16

FILL_NS = 450         # filler PE-ns drained per attention step


def _block_structure(mask: np.ndarray):
    """Classify maskT (k,q) blocks: per q-tile a list of (kt, pattern_idx|None).

    For each unique partial pattern also derive (w0, m_lo, m_hi): w0 leading
    all-masked columns (exp skipped, memset 0), and [m_lo, m_hi) the column
    range that still needs the mask multiply.
    """
    maskT = (mask != 0).T.astype(np.float32)  # [k, q] visibility
    vis = []
    patterns = []
    meta = []
    pat_index = {}
    for qt in range(NQ):
        row = []
        for kt in range(NKT):
            blk = maskT[kt * P:(kt + 1) * P, qt * QT:(qt + 1) * QT]
            s = blk.sum()
            if s == 0:
                continue
            if s == blk.size:
                row.append((kt, None))
            else:
                key = blk.tobytes()
                if key not in pat_index:
                    pat_index[key] = len(patterns)
                    patterns.append(blk)
                    col_any = blk.any(axis=0)       # column has any visible
                    col_all = blk.all(axis=0)       # column fully visible
                    w0 = int(np.argmax(col_any)) if col_any.any() else QT
                    partial_cols = np.nonzero(col_any & ~col_all)[0]
                    if partial_cols.size:
                        m_lo, m_hi = int(partial_cols[0]), int(partial_cols[-1]) + 1
                    else:
                        m_lo = m_hi = 0
                    meta.append((w0, m_lo, m_hi))
                row.append((kt, pat_index[key]))
        vis.append(row)
    if patterns:
        pm = np.stack(patterns)
    else:
        pm = np.zeros((1, P, QT), np.float32)
    return vis, pm, meta


def _build_program(vis, n_pm, meta=(), compile=True):
    nc = bacc.Bacc() if compile else bass.Bass()
    # host-pre-swizzled layouts: every dram row (dim0) maps to one partition
    # and is contiguous, so DMAs are 128 plain descriptors.
    xs = nc.declare_dram_parameter("xs", [P, NKT * ND * P], CDT, isOutput=False)
    wq = nc.declare_dram_parameter("wq", [P, ND * HPC * Dh], CDT, isOutput=False)
    wk = nc.declare_dram_parameter("wk", [P, ND * HPC * Dh], CDT, isOutput=False)
    wv = nc.declare_dram_parameter("wv", [P, ND * HPC * Dh], CDT, isOutput=False)
    wo = nc.declare_dram_parameter("wo", [P, NPAIR * D], CDT, isOutput=False)
    pmask = nc.declare_dram_parameter("pmask", [P, n_pm * 2 * QT], CDT, isOutput=False)
    y = nc.declare_dram_parameter("y", [T, D], CDT, isOutput=True)

    inv_sqrt_dh = 1.0 / float(np.sqrt(Dh))

    with tile.TileContext(nc) as tc:
        with (
            tc.tile_pool(name="persist", bufs=1) as persist,
            tc.tile_pool(name="work", bufs=3) as work,
            tc.tile_pool(name="epi", bufs=2) as epi,
            tc.tile_pool(name="psA", bufs=2, space="PSUM") as psA,
            tc.tile_pool(name="psS", bufs=2, space="PSUM") as psS,
            tc.tile_pool(name="psU", bufs=1, space="PSUM") as psU,
        ):
            # ---- persistent SBUF tensors ----
            # x in t-128-block-major layout: (p, tt, o, t') so both the
            # 128-wide v stationaries and DMA staging slices are contiguous
            xt_sb = persist.tile([P, NKT, ND, P], CDT, tag="xt")
            wq_sb = persist.tile([P, ND, HPC * Dh], CDT, tag="wq")
            wk_sb = persist.tile([P, ND, HPC * Dh], CDT, tag="wk")
            wv_sb = persist.tile([P, ND, HPC * Dh], CDT, tag="wv")
            wo_sb = persist.tile([P, NPAIR, D], CDT, tag="wo")
            pm_sb = persist.tile([P, n_pm, 2, QT], CDT, tag="pm")
            qT_sb = [persist.tile([P, T], CDT, tag=f"qT{p}", name=f"qT{p}") for p in range(NPAIR)]
            kT_sb = [persist.tile([P, T], CDT, tag=f"kT{p}", name=f"kT{p}") for p in range(NPAIR)]
            at_sb = [persist.tile([P, T], CDT, tag=f"at{p}", name=f"at{p}") for p in range(NPAIR)]
            # v1: per k-tile and head, [128, 128]: for even heads cols 0:64 =
            # v values and cols 64:128 all-ones (for odd heads the reverse),
            # so the attnU matmul emits softmax denominators replicated on the
            # complementary partition half (matmul cost only depends on N).
            v1_sb = persist.tile([P, NKT, HPC, P], CDT, tag="v1")

            # ones blocks (v halves overwritten later); per 4-kt group so the
            # first v evictions don't wait on the whole memset
            for g in range(4):
                nc.gpsimd.memset(v1_sb[:, g * 4:(g + 1) * 4], 1.0)

            # small leading pieces so the first v-projection matmuls can start
            # as soon as ~0.5MB has landed, instead of waiting for full tiles
            wvr = wv.rearrange("p (o e) -> p o e", o=ND)
            xr = xs.rearrange("p (b o t) -> p b o t", b=NKT, o=ND)
            nc.sync.dma_start(wv_sb[:, 0:4], wvr[:, 0:4])
            nc.sync.dma_start(xt_sb[:, 0:1], xr[:, 0:1])
            nc.sync.dma_start(wv_sb[:, 4:ND], wvr[:, 4:ND])
            nc.sync.dma_start(xt_sb[:, 1:4], xr[:, 1:4])
            nc.sync.dma_start(wq_sb[:], wq.rearrange("p (o e) -> p o e", o=ND))
            nc.sync.dma_start(wk_sb[:], wk.rearrange("p (o e) -> p o e", o=ND))
            for c in range(1, NQ):
                nc.sync.dma_start(xt_sb[:, c * 4:(c + 1) * 4], xr[:, c * 4:(c + 1) * 4])
            nc.sync.dma_start(wo_sb[:], wo.rearrange("p (r e) -> p r e", r=NPAIR))
            nc.sync.dma_start(pm_sb[:], pmask.rearrange("p (n h q) -> p n h q", n=n_pm, h=2))

            # ---- filler queue: PE-side work interleaved into ACT-paced ----
            # ---- attention steps. entries: (key, pe_cost_ns, thunk)      ----
            fillers = []
            budget = [0.0]

            def drain_budget(extra_ns):
                budget[0] += extra_ns
                while fillers and budget[0] >= fillers[0][1]:
                    key, cost, thunk = fillers.pop(0)
                    budget[0] -= cost
                    thunk()

            def flush(pred):
                """Emit and remove exactly the queued fillers matching pred."""
                keep = []
                for ent in fillers:
                    if pred(ent[0]):
                        ent[2]()
                    else:
                        keep.append(ent)
                fillers[:] = keep

            # ---- phase A: v = x @ wv for one k-tile ----
            def emit_v(tt):
                ps_v = psA.tile([P, QT], f32, tag="psA", name=f"psv{tt}")
                for dt in range(ND):
                    nc.tensor.matmul(
                        ps_v[:, :HPC * Dh],
                        xt_sb[:, tt, dt, :],
                        wv_sb[:, dt, :],
                        start=(dt == 0),
                        stop=(dt == ND - 1),
                    )
                ps_vh = ps_v[:, :HPC * Dh].rearrange("p (h e) -> p h e", h=HPC)
                nc.vector.tensor_copy(v1_sb[:, tt, 0::2, 0:Dh], ps_vh[:, 0::2])
                nc.vector.tensor_copy(v1_sb[:, tt, 1::2, Dh:P], ps_vh[:, 1::2])

            # ---- phase A: qT / kT projection unit for (pair, tensor, nt) ----
            def make_qk(p, which, nt):
                w_sb = wq_sb if which == "q" else wk_sb
                out_sb = qT_sb[p] if which == "q" else kT_sb[p]

                def go():
                    ps = psA.tile([P, QT], f32, tag="psA", name=f"psqk{p}{which}{nt}")
                    for dt in range(ND):
                        nc.tensor.matmul(
                            ps,
                            w_sb[:, dt, p * P:(p + 1) * P],
                            xt_sb[:, nt * 4:(nt + 1) * 4, dt, :],
                            start=(dt == 0),
                            stop=(dt == ND - 1),
                        )
                    nc.vector.tensor_copy(out_sb[:, nt * QT:(nt + 1) * QT], ps)
                return go

            # ---- phase C: out-projection for one full t-tile (as filler) ----
            # both 512-halves in one unit -> one contiguous-row y DMA per tt;
            # evictions alternate DVE / ScalarE so neither paces the tail
            def make_outproj(tt):
                def go():
                    ysb = work.tile([P, 2, QT], CDT, tag="y", name=f"y{tt}")
                    for half in range(2):
                        ps_y = psA.tile([P, QT], f32, tag="psA", name=f"psy{tt}_{half}")
                        for p in range(NPAIR):
                            nc.tensor.matmul(
                                ps_y[:],
                                at_sb[p][:, tt * P:(tt + 1) * P],
                                wo_sb[:, p, half * QT:(half + 1) * QT],
                                start=(p == 0),
                                stop=(p == NPAIR - 1),
                            )
                        if half == 0:
                            nc.vector.tensor_copy(ysb[:, half, :], ps_y[:])
                        else:
                            nc.scalar.copy(ysb[:, half, :], ps_y[:])
                    nc.sync.dma_start(
                        y[tt * P:(tt + 1) * P, :].rearrange("t (h q) -> t h q", h=2),
                        ysb[:])
                return go

            # ---- phase B: attention row (pair p, q-tile qt) ----
            def attention_row(p, qt):
                row = vis[qt]
                flush(lambda k: k[0] == "qk" and k[1] == p and k[2] == "q" and k[3] == qt)
                if not row:
                    nc.vector.memset(at_sb[p][:, qt * QT:(qt + 1) * QT], 0.0)
                    return
                ps_u = psU.tile([P, 2, QT], f32, tag="u", name=f"u_{p}_{qt}")
                es_q = []

                def emit_scores(j):
                    kt, pidx = row[j]
                    flush(lambda k: k[0] == "qk" and k[1] == p and k[2] == "k"
                          and k[3] <= kt // 4)
                    w0 = 0 if pidx is None else meta[pidx][0]
                    if j == 0:
                        w0 = 0  # first accumulation must set has_written
                    ps_s = psS.tile([P, 2, QT], f32, tag="s", name=f"s_{p}_{qt}_{kt}")
                    for h in range(2):
                        base = h * Dh
                        nc.tensor.matmul(
                            ps_s[:, h, w0:QT],
                            kT_sb[p][base:base + Dh, kt * P:(kt + 1) * P],
                            qT_sb[p][base:base + Dh, qt * QT + w0:(qt + 1) * QT],
                            start=True,
                            stop=True,
                            tile_position=(base, 0),
                        )
                    es = work.tile([P, 2, QT], CDT, tag="es", name=f"es_{p}_{qt}_{kt}")
                    nc.scalar.activation(
                        es[:, :, w0:QT], ps_s[:, :, w0:QT],
                        mybir.ActivationFunctionType.Exp,
                        scale=inv_sqrt_dh,
                    )
                    if pidx is not None:
                        _w0, m_lo, m_hi = meta[pidx]
                        m_lo = min(m_lo, w0)  # w0 forced to 0 on j==0
                        if m_hi > m_lo:
                            nc.gpsimd.tensor_mul(
                                es[:, :, m_lo:m_hi], es[:, :, m_lo:m_hi],
                                pm_sb[:, pidx, :, m_lo:m_hi],
                            )
                    es_q.append((es, w0))

                def emit_attnu(j):
                    kt, _ = row[j]
                    flush(lambda k: k[0] == "v" and k[1] == kt)
                    es, w0 = es_q[j]
                    for h in range(2):
                        nc.tensor.matmul(
                            ps_u[:, h, w0:QT],
                            v1_sb[:, kt, 2 * p + h, :],
                            es[:, h, w0:QT],
                            start=(j == 0),
                            stop=(j == len(row) - 1),
                        )

                emit_scores(0)
                if len(row) > 1:
                    emit_scores(1)
                for j in range(len(row)):
                    drain_budget(FILL_NS)
                    if j + 2 < len(row):
                        emit_scores(j + 2)
                    emit_attnu(j)

                # Stage both ps_u halves to SBUF with one ScalarE ACT copy per
                # head (Copy shares the exp ACT table -> no table reload).
                # This releases the PSUM banks ~1.3us after the row's last
                # attnU, so the next row's attnU(0) (psU bufs=1 WAR) never
                # stalls the PE on the divide chain below.
                stg = [epi.tile([P, QT], f32, tag=f"stg{h}", name=f"stg{h}_{p}_{qt}")
                       for h in range(2)]
                for h in range(2):
                    nc.scalar.copy(stg[h][:], ps_u[:, h, :])
                for h in range(2):
                    # staged[:, h]: for h==0 partitions 0:64 = attn, 64:128 =
                    # denominators replicated (ones half of v1); for h==1 the
                    # reverse. The custom-DVE reciprocal only works at base
                    # partition 0 on HW, so denominators go to partitions 0:64.
                    a_sl = slice(0, Dh) if h == 0 else slice(Dh, P)
                    if h == 0:
                        sums2 = epi.tile([P, QT], f32, tag="sums2", name=f"s2_{h}_{p}_{qt}")
                        nc.sync.dma_start(sums2[0:Dh, :], stg[0][Dh:P, :])
                        rsrc = sums2
                    else:
                        rsrc = stg[1]
                    rep = epi.tile([P, QT], f32, tag="rep", name=f"rep{h}_{p}_{qt}")
                    nc.vector.reciprocal_approx_fast(rep[0:Dh, :], rsrc[0:Dh, :])
                    if h == 0:
                        mul_in1 = rep[0:Dh, :]
                    else:
                        rep2 = epi.tile([P, QT], f32, tag="rep2", name=f"rp2_{h}_{p}_{qt}")
                        nc.sync.dma_start(rep2[Dh:P, :], rep[0:Dh, :])
                        mul_in1 = rep2[Dh:P, :]
                    nc.vector.tensor_mul(
                        at_sb[p][h * Dh:(h + 1) * Dh, qt * QT:(qt + 1) * QT],
                        stg[h][a_sl, :],
                        mul_in1,
                    )
                if p == NPAIR - 1:
                    fillers.extend(
                        (("op", tt), 1000, make_outproj(tt))
                        for tt in range(qt * (QT // P), (qt + 1) * (QT // P))
                    )

            # inline prologue: v for the first q-tile's k range
            for tt in range(4):
                emit_v(tt)
            fillers.extend((("v", tt), 860, (lambda tt=tt: emit_v(tt)))
                           for tt in range(4, NKT))
            for p in range(NPAIR):
                fillers.extend((("qk", p, which, nt), 1750, make_qk(p, which, nt))
                               for which in ("q", "k") for nt in range(NQ))

            for qt in range(NQ):
                for p in range(NPAIR):
                    attention_row(p, qt)
            flush(lambda k: True)
    if compile:
        nc.compile()
    return nc


def _host_inputs(x, mask, w_qkv, w_out):
    vis, pm, meta = _block_structure(np.asarray(mask))
    # pmask duplicated along an h axis so one multiply covers the merged
    # two-head es tile: [P, n_pm, 2, QT]
    pm_h = np.broadcast_to(
        pm.transpose(1, 0, 2)[:, :, None, :], (P, pm.shape[0], 2, QT)
    ).reshape(P, -1).astype(NP_CDT)
    pm_h = np.ascontiguousarray(pm_h)
    wq_f, wk_f, wv_f = np.split(np.asarray(w_qkv, np.float32), 3, axis=1)

    def swz_w(w):  # [1024, E] -> [128, 8*E] with row (o*128+p) at (p, o)
        e = w.shape[1]
        return np.ascontiguousarray(
            w.reshape(ND, P, e).transpose(1, 0, 2).reshape(P, ND * e)
        ).astype(NP_CDT)

    in_maps = []
    for core in range(N_CORES):
        b = core // 4
        g = core % 4
        cols = slice(g * HPC * Dh, (g + 1) * HPC * Dh)
        xT = np.asarray(x[b], np.float32).T  # [D, T]
        # (p, tt, o, t'): xs[p, tt, o, t'] = xT[o*128+p, tt*128+t']
        xs_s = np.ascontiguousarray(
            xT.reshape(ND, P, NKT, P).transpose(1, 2, 0, 3).reshape(P, -1)
        ).astype(NP_CDT)
        wo_c = np.asarray(w_out, np.float32)[cols, :]  # [256, 1024]
        wo_s = np.ascontiguousarray(
            wo_c.reshape(NPAIR, P, D).transpose(1, 0, 2).reshape(P, NPAIR * D)
        ).astype(NP_CDT)
        in_maps.append({
            "xs": xs_s,
            "wq": swz_w(wq_f[:, cols]),
            "wk": swz_w(wk_f[:, cols]),
            "wv": swz_w(wv_f[:, cols]),
            "wo": wo_s,
            "pmask": pm_h,
        })
    return vis, pm, meta, in_maps


def run(x, mask, w_qkv, w_out, trace=False):
    import os
    vis, pm, meta, in_maps = _host_inputs(x, mask, w_qkv, w_out)
    nc = _build_program(vis, pm.shape[0], meta)
    if not trace:
        # An inherited BASS_TRACE=1 would pull in NTFF profiling hooks that
        # may not exist in this environment; force tracing off.
        os.environ["BASS_NEVER_TRACE"] = "1"
    else:
        os.environ.pop("BASS_NEVER_TRACE", None)
    res = run_bass_kernel_spmd(nc, in_maps, core_ids=list(range(N_CORES)), trace=trace)
    parts = [res.results[i]["y"].astype(np.float32) for i in range(N_CORES)]
    out = np.stack([
        parts[0] + parts[1] + parts[2] + parts[3],
        parts[4] + parts[5] + parts[6] + parts[7],
    ]).astype(np.float32)
    return out, res


def kernel(x, mask, w_qkv, w_out):
    out, _ = run(x, mask, w_qkv, w_out, trace=False)
    return out
